# revision 26
# baseline (speedup 1.0000x reference)
"""Trainium2 Bass kernel for MultiHeadAttentionWithRope.

Problem: B=2, T=2048, C=2048, H=16 heads, D=128 head_dim, fp32 I/O.
  qkv = x @ W_qkv; q,k -> RoPE (adjacent-pair, torchtune) -> causal SDPA
  -> out = o @ W_out
Sharding (8 cores): 2 batches x 4 head-groups (4 heads each); host sums the
4 row-shard partials of W_out per batch.

Numerics/performance scheme (per-core PE time is the binding constraint):
  - all dense GEMMs in fp16 (same PE rate as bf16, 8x less noise)
  - softmax probs pt stored fp8e4 with a constant exp bias (shift cancels in
    the l-normalization): the ONLY tensor whose fp8 noise the output metric
    tolerates (renormalization cancels the dominant-prob error)
  - row-sums l via an all-ones fp8 DoubleRow matmul over k-chunk PAIRS:
    half the PE cost of the bf16 row-sum matmul
  - PV via TWO fp8 DoubleRow matmuls per chunk-pair with v stored as an
    fp8 hi+lo pair (v_hi + v_lo == v to ~0.1%): bf16-parity cost, keeps
    v effectively exact while consuming the shared fp8 pt
  - causal masking via affine_select on the otherwise-idle Pool engine,
    zeroing the invalid triangle of the fp8 probs after exp
  - phase-2 software pipelining: scores/exp run ~3 chunks ahead of the
    DR consumers; out-proj tiles drip-fed between pairs to fill PE waits
"""

import sys

sys.path.insert(0, "/opt/trn_rl_repo")

import numpy as np
import ml_dtypes

import concourse.bass as bass
import concourse.tile as tile
from concourse import mybir
from concourse.bass import ts
from concourse.bass_utils import run_bass_kernel_spmd

# Provide antenv.axon_hooks (absent in this container) so trace=True can use
# the axon NTFF profiling path.
def _ensure_axon_hooks():
    import types

    try:
        from antenv import axon_hooks  # noqa: F401
        return
    except ImportError:
        pass
    import antenv

    mod = types.ModuleType("antenv.axon_hooks")
    mod._hook = None

    def set_axon_ntff_profile_hook(h):
        mod._hook = h

    def get_axon_ntff_profile_hook():
        return mod._hook

    mod.set_axon_ntff_profile_hook = set_axon_ntff_profile_hook
    mod.get_axon_ntff_profile_hook = get_axon_ntff_profile_hook
    sys.modules["antenv.axon_hooks"] = mod
    antenv.axon_hooks = mod
    try:
        from trn_agent_boot.trn_boot import _ntff_profile_via_ctypes

        hook = _ntff_profile_via_ctypes("/opt/axon/libaxon_pjrt.so")
        if hook is not None:
            mod._hook = hook
    except Exception:
        pass


_ensure_axon_hooks()

# ---------------------------------------------------------------------------
# This walrus build supports only ONE sync-wait command per instruction.
# TileContext's sem assignment can attach several waits to one instruction
# (and its exit drain aggregates many). Post-pass: hoist excess waits onto
# same-engine NoOps inserted immediately before the instruction -- the
# engine blocks on each wait in order, so semantics are identical.
MAX_WAITS_PER_INST = 1


_ALL_ENGINES = [
    mybir.EngineType.PE,
    mybir.EngineType.Activation,
    mybir.EngineType.DVE,
    mybir.EngineType.Pool,
    mybir.EngineType.SP,
]


def _split_sync_waits(nc):
    for f in nc.m.functions:
        for blk in f.blocks:
            new_insts = []
            for ins in blk.instructions:
                si = getattr(ins, "sync_info", None)
                lim = 1 if isinstance(ins, mybir.InstDrain) else MAX_WAITS_PER_INST
                if si is not None and si.on_wait and len(si.on_wait) > lim:
                    waits = list(si.on_wait)
                    keep = waits[:lim]
                    extra = waits[lim:]
                    # A drain with a big wait-set is the kernel-tail barrier:
                    # spread its waits across all engines so they resolve in
                    # parallel (the all-engine barrier right after joins them).
                    spread = (
                        isinstance(ins, mybir.InstDrain) and len(extra) > 4
                    )
                    for i, w in enumerate(extra):
                        eng = (
                            _ALL_ENGINES[i % len(_ALL_ENGINES)]
                            if spread
                            else ins.engine
                        )
                        nop = mybir.InstNoOp(
                            name=nc.get_next_instruction_name(),
                            sync_info=mybir.SyncInfo(on_wait=[w], on_update=[]),
                            bass_nofuse=True,
                            engine=eng,
                        )
                        new_insts.append(nop)
                    si.on_wait = keep
                new_insts.append(ins)
            if len(new_insts) != len(blk.instructions):
                blk.instructions = new_insts


# ---------------------------------------------------------------------------

B, T, C, H = 2, 2048, 2048, 16
D = C // H  # 128
ROPE_BASE = 10000.0
HG = 4  # head groups
HL = H // HG  # heads per core = 4
CL = HL * D  # local width = 512
P = 128
TB = 512  # token block
NTB = T // TB  # 4
KCH = T // P  # 16 k-chunks
QT = T // TB  # 4 q-tiles
SCALE = 1.0 / float(np.sqrt(D))
EXP_BIAS = -1.0  # exp shift; cancels in normalization, keeps pt under fp8e4 max
MASKNEG = -30000.0

F16 = mybir.dt.float16
FP8 = mybir.dt.float8e4
F32 = mybir.dt.float32
DR = mybir.MatmulPerfMode.DoubleRow
f16_np = np.float16
fp8_np = ml_dtypes.float8_e4m3


def _build_nc():
    nc = bass.Bass(trn_type="TRN2")
    xT = nc.declare_dram_parameter("xT", [C, T], F16, isOutput=False)
    wqkv = nc.declare_dram_parameter("wqkv", [C, 3 * CL], F16, isOutput=False)
    wout = nc.declare_dram_parameter("wout", [CL, C], F16, isOutput=False)
    tabs = nc.declare_dram_parameter("tabs", [2, P, T], F16, isOutput=False)
    out = nc.declare_dram_parameter("out", [T, C], F16, isOutput=True)

    xT_r = xT[:].rearrange("(ko p) t -> p ko t", p=P)  # [128,16,T]
    wqkv_r = wqkv[:].rearrange("(ko p) m -> p ko m", p=P)  # [128,16,1536]
    wout_r = wout[:].rearrange("(h p) n -> p h n", p=P)  # [128,4,2048]
    out_r = out[:].rearrange("(tc p) n -> tc p n", p=P)  # [16,128,2048]

    with tile.TileContext(nc) as tc:
        consts = tc.alloc_tile_pool(name="consts", bufs=1)
        persist = tc.alloc_tile_pool(name="persist", bufs=1)
        xpool = tc.alloc_tile_pool(name="xpool", bufs=2)
        prepool = tc.alloc_tile_pool(name="prepool", bufs=3)
        swppool = tc.alloc_tile_pool(name="swppool", bufs=4)
        ropepool = tc.alloc_tile_pool(name="ropepool", bufs=3)
        ppool = tc.alloc_tile_pool(name="ppool", bufs=4)
        vtmppool = tc.alloc_tile_pool(name="vtmppool", bufs=3)
        rpool = tc.alloc_tile_pool(name="rpool", bufs=2)
        outpool = tc.alloc_tile_pool(name="outpool", bufs=4)
        mm_psum = tc.alloc_tile_pool(name="mm_psum", bufs=2, space="PSUM")
        acc_psum = tc.alloc_tile_pool(name="acc_psum", bufs=3, space="PSUM")

        # ---- HAM warmup ----
        # PE is DMA-starved at start and its HAM clock gate starts cold
        # (1.2 GHz, ~3.4us to warm). Run dummy matmuls on scratch data during
        # the wait so real matmuls start at 2.4 GHz.
        warm_sb = prepool.tile([P, TB], F16, tag="pre")
        nc.vector.memset(warm_sb[:], 1.0)
        warm_ps = mm_psum.tile([P, 2, TB], F32, tag="mm")
        for _ in range(12):
            nc.tensor.matmul(
                warm_ps[:, 0, :], lhsT=warm_sb[:, :P], rhs=warm_sb[:],
                start=True, stop=True,
            )
        warm_out = prepool.tile([P, TB], F16, tag="pre")
        nc.scalar.copy(out=warm_out[:], in_=warm_ps[:, 0, :])

        # ---- constants ----
        # DMA issue order is program order; the first matmuls need only the
        # kc=0 slices of xT(tb=0) and W, so interleave those first.
        w_sb = consts.tile([P, KCH, 3 * CL], F16)  # 6MB
        xt0 = xpool.tile([P, KCH, TB], F16, tag="xt")
        for kc in range(KCH):
            nc.sync.dma_start(out=xt0[:, kc, :], in_=xT_r[:, kc, ts(0, TB)])
            nc.sync.dma_start(out=w_sb[:, kc, :], in_=wqkv_r[:, kc, :])
        bias_sb = consts.tile([P, 1], F32)
        nc.vector.memset(bias_sb[:], EXP_BIAS)
        cos_sb = consts.tile([P, T], F16)
        nc.sync.dma_start(out=cos_sb[:], in_=tabs[0])
        sin_sb = consts.tile([P, T], F16)
        nc.sync.dma_start(out=sin_sb[:], in_=tabs[1])
        ones_sb = consts.tile([P, 2, P], FP8)
        nc.vector.memset(ones_sb[:], 1.0)
        wo_sb = consts.tile([P, HL, C], F16)  # 2MB, loaded later (phase 3 use)

        # ---- persistent activations ----
        qT_sb = persist.tile([P, HL, T], F16)  # 2MB
        kT_sb = persist.tile([P, HL, T], F16)  # 2MB
        vhi_sb = persist.tile([P, KCH, CL], FP8)  # 1MB
        vlo_sb = persist.tile([P, KCH, CL], FP8)  # 1MB
        oT_sb = persist.tile([P, HL, T], F16)  # 2MB

        # ================= Phase 1: QKV + RoPE =================
        for tb in range(NTB):
            if tb == 0:
                xt = xt0
            else:
                xt = xpool.tile([P, KCH, TB], F16, tag="xt")
                nc.sync.dma_start(out=xt[:], in_=xT_r[:, :, ts(tb, TB)])

            # two accumulation chains share each 2-bank psum tile
            chain_tile = [None]

            def chain_psum(ch):
                if ch % 2 == 0:
                    chain_tile[0] = mm_psum.tile(
                        [P, 2, TB], F32, tag="mm", name="qp"
                    )
                return chain_tile[0][:, ch % 2, :]

            # qT / kT in [D, token] layout + RoPE
            for m in range(2 * HL):  # 0..3 q heads, 4..7 k heads
                qp = chain_psum(m)
                for kc in range(KCH):
                    nc.tensor.matmul(
                        qp[:],
                        lhsT=w_sb[:, kc, ts(m, P)],
                        rhs=xt[:, kc, :],
                        start=(kc == 0),
                        stop=(kc == KCH - 1),
                    )
                pre = prepool.tile([P, TB], F16, tag="pre")
                nc.scalar.copy(out=pre[:], in_=qp[:])
                del qp
                # swap partition halves via SBUF->SBUF DMA (DVE cannot cross
                # partitions; walrus requires equal base partitions for TT)
                h64 = D // 2
                swp = swppool.tile([P, TB], F16)
                nc.sync.dma_start(out=swp[0:h64], in_=pre[h64 : 2 * h64])
                nc.sync.dma_start(out=swp[h64 : 2 * h64], in_=pre[0:h64])
                ta = ropepool.tile([P, TB], F16, tag="ta")
                tb_ = ropepool.tile([P, TB], F16, tag="tb")
                # rope = pre * cosF + swap(pre) * sinS   (sinS = [-sin; +sin])
                nc.vector.tensor_mul(ta[:], pre[:], cos_sb[:, ts(tb, TB)])
                nc.vector.tensor_mul(tb_[:], swp[:], sin_sb[:, ts(tb, TB)])
                dest = (
                    qT_sb[:, m, ts(tb, TB)] if m < HL else kT_sb[:, m - HL, ts(tb, TB)]
                )
                nc.vector.tensor_add(dest[:], ta[:], tb_[:])

            # V in natural [token, D] layout; one fast ACT copy frees the
            # psum slot, then the fp8 hi+lo split happens SBUF-side on DVE
            for tsc in range(TB // P):
                vp = chain_psum(2 * HL + tsc)
                for kc in range(KCH):
                    nc.tensor.matmul(
                        vp[:],
                        lhsT=xt[:, kc, ts(tsc, P)],
                        rhs=w_sb[:, kc, 2 * CL : 3 * CL],
                        start=(kc == 0),
                        stop=(kc == KCH - 1),
                    )
                vslot = tb * (TB // P) + tsc
                v16 = vtmppool.tile([P, TB], F16, tag="v16")
                nc.scalar.copy(out=v16[:], in_=vp[:])
                nc.vector.tensor_copy(out=vhi_sb[:, vslot, :], in_=v16[:])
                nc.vector.tensor_sub(
                    vlo_sb[:, vslot, :], v16[:], vhi_sb[:, vslot, :]
                )

        # ================= Phase 2+3: attention (qt-outer) + out-proj =====
        # this load overlaps phase 1/2 compute
        nc.sync.dma_start(out=wo_sb[:], in_=wout_r)

        out_units = []  # (tcc, ncc) out-proj tiles ready to emit

        def emit_out_unit():
            if not out_units:
                return
            tcc, ncc = out_units.pop(0)
            outp = mm_psum.tile([P, TB], F32, tag="outp", bufs=1)
            for h in range(HL):
                nc.tensor.matmul(
                    outp[:],
                    lhsT=oT_sb[:, h, ts(tcc, P)],
                    rhs=wo_sb[:, h, ts(ncc, TB)],
                    start=(h == 0),
                    stop=(h == HL - 1),
                )
            ot = outpool.tile([P, TB], F16)
            nc.vector.tensor_copy(out=ot[:], in_=outp[:])
            nc.sync.dma_start(out=out_r[tcc, :, ts(ncc, TB)], in_=ot[:])

        def emit_attention(qt, h):
            op = acc_psum.tile([P, TB], F32, tag="acc")
            lp = acc_psum.tile([P, TB], F32, tag="acc")
            nkc = (qt + 1) * (TB // P)
            npair = nkc // 2
            pts = {}
            sps = {}

            def pair_geom(p_):
                off_e = 2 * p_ - qt * (TB // P)
                qs = max(off_e, 0) * P
                return off_e >= 0, qs, TB - qs

            def produce(c):
                p_ = c // 2
                diag, qs, W = pair_geom(p_)
                if c % 2 == 0:
                    sps[p_] = mm_psum.tile([P, 2, TB], F32, tag="mm", name="sp")
                    pts[p_] = ppool.tile([P, 2, TB], FP8, name="pt")
                sp = sps[p_]
                nc.tensor.matmul(
                    sp[:, c % 2, :W],
                    lhsT=kT_sb[:, h, ts(c, P)],
                    rhs=qT_sb[:, h, qt * TB + qs : (qt + 1) * TB],
                    start=True,
                    stop=True,
                )
                if c % 2 == 1:
                    # one exp covers the whole chunk-pair
                    nc.scalar.activation(
                        out=pts[p_][:, :, :W],
                        in_=sp[:, :, :W],
                        func=mybir.ActivationFunctionType.Exp,
                        scale=SCALE,
                        bias=bias_sb[:],
                    )
                    sps.pop(p_)
                    if diag:
                        # zero the causally-invalid triangle on the idle Pool
                        # engine: keep where (c_local - 128*subtile - k) >= 0
                        nc.gpsimd.affine_select(
                            out=pts[p_][:, :, :W],
                            in_=pts[p_][:, :, :W],
                            pattern=[[-P, 2], [1, W]],
                            compare_op=mybir.AluOpType.is_ge,
                            fill=0.0,
                            base=0,
                            channel_multiplier=-1,
                        )

            def consume(p_):
                _, qs, W = pair_geom(p_)
                kc = 2 * p_
                pt = pts.pop(p_)
                last = p_ == npair - 1
                nc.tensor.matmul(
                    op[:, qs:],
                    lhsT=vhi_sb[:, kc : kc + 2, ts(h, P)],
                    rhs=pt[:, :, :W],
                    start=(p_ == 0),
                    stop=False,
                    perf_mode=DR,
                )
                nc.tensor.matmul(
                    op[:, qs:],
                    lhsT=vlo_sb[:, kc : kc + 2, ts(h, P)],
                    rhs=pt[:, :, :W],
                    start=False,
                    stop=last,
                    perf_mode=DR,
                )
                nc.tensor.matmul(
                    lp[:, qs:],
                    lhsT=ones_sb[:],
                    rhs=pt[:, :, :W],
                    start=(p_ == 0),
                    stop=last,
                    perf_mode=DR,
                )

            # scores/exp run ~3 chunks ahead of the DR consumers so exp
            # latency hides behind PE work; out-proj drips fill the rest
            done = 0
            for c in range(nkc):
                produce(c)
                if c >= 3 and c % 2 == 1:
                    consume(done)
                    done += 1
                    emit_out_unit()
            while done < npair:
                consume(done)
                done += 1

            # r = 1/l = exp(-ln(l)); fold into oT
            lt = rpool.tile([P, TB], F32, tag="lt")
            nc.scalar.activation(
                out=lt[:], in_=lp[:], func=mybir.ActivationFunctionType.Ln
            )
            rt = rpool.tile([P, TB], F32, tag="rt")
            nc.scalar.activation(
                out=rt[:],
                in_=lt[:],
                func=mybir.ActivationFunctionType.Exp,
                scale=-1.0,
            )
            nc.vector.tensor_mul(oT_sb[:, h, ts(qt, TB)], op[:], rt[:])

        for qt in range(QT):
            for h in range(HL):
                emit_attention(qt, h)
                emit_out_unit()
                emit_out_unit()
            out_units.extend(
                (tcc, ncc)
                for tcc in range(qt * (TB // P), (qt + 1) * (TB // P))
                for ncc in range(C // TB)
            )
        while out_units:
            emit_out_unit()

        for pool in (
            acc_psum,
            mm_psum,
            outpool,
            rpool,
            vtmppool,
            ppool,
            ropepool,
            swppool,
            prepool,
            xpool,
            persist,
            consts,
        ):
            pool.release()

    _split_sync_waits(nc)
    return nc


def _host_inputs(x, W_qkv, W_out):
    """Build per-core input maps. Core j: batch j//HG, head-group j%HG."""
    perm = np.concatenate([np.arange(0, D, 2), np.arange(1, D, 2)])  # deinterleave

    # rope tables in de-interleaved layout: rows [0:64]=even-dim freq, dup below
    inv = 1.0 / (ROPE_BASE ** (np.arange(0, D, 2, dtype=np.float32) / D))  # [64]
    ang = np.arange(T, dtype=np.float32)[None, :] * inv[:, None]  # [64, T]
    cosF = np.concatenate([np.cos(ang), np.cos(ang)], axis=0)  # [128, T]
    sinS = np.concatenate([-np.sin(ang), np.sin(ang)], axis=0)  # sign folded
    tabs = np.stack([cosF, sinS]).astype(f16_np)  # [2,128,T]

    in_maps = []
    for j in range(8):
        b, hg = j // HG, j % HG
        xTb = np.ascontiguousarray(x[b].T).astype(f16_np)  # [C, T]
        cols = []
        for part in range(2):  # q, k with permuted D
            for h in range(HL):
                base = part * C + (hg * HL + h) * D
                cols.append(W_qkv[:, base + perm])
        for h in range(HL):  # v natural
            base = 2 * C + (hg * HL + h) * D
            cols.append(W_qkv[:, base : base + D])
        wq = np.concatenate(cols, axis=1).astype(f16_np)  # [C, 3*CL]
        wo = W_out[hg * CL : (hg + 1) * CL, :].astype(f16_np)  # [CL, C]
        in_maps.append({"xT": xTb, "wqkv": wq, "wout": wo, "tabs": tabs})
    return in_maps


def kernel(x, W_qkv, W_out, _trace=False, _tmpdir=None):
    x = np.asarray(x, dtype=np.float32)
    W_qkv = np.asarray(W_qkv, dtype=np.float32)
    W_out = np.asarray(W_out, dtype=np.float32)

    nc = _build_nc()
    in_maps = _host_inputs(x, W_qkv, W_out)
    res = run_bass_kernel_spmd(
        nc, in_maps, core_ids=list(range(8)), trace=_trace, tmpdir=_tmpdir
    )

    out = np.zeros((B, T, C), dtype=np.float32)
    for j in range(8):
        out[j // HG] += res.results[j]["out"].astype(np.float32)
    if _trace:
        return out, res
    return out


# revision 29
# speedup vs baseline: 1.0808x; 1.0808x over previous
"""Trainium2 Bass kernel for MultiHeadAttentionWithRope.

Problem: B=2, T=2048, C=2048, H=16 heads, D=128 head_dim, fp32 I/O.
  qkv = x @ W_qkv; q,k -> RoPE (adjacent-pair, torchtune) -> causal SDPA
  -> out = o @ W_out
Sharding (8 cores): 2 batches x 4 head-groups (4 heads each); host sums the
4 row-shard partials of W_out per batch.

Numerics/performance scheme (per-core PE time is the binding constraint):
  - all dense GEMMs in fp16 (same PE rate as bf16, 8x less noise)
  - softmax probs pt stored fp8e4 with a constant exp bias (shift cancels in
    the l-normalization): the ONLY tensor whose fp8 noise the output metric
    tolerates (renormalization cancels the dominant-prob error)
  - row-sums l via an all-ones fp8 DoubleRow matmul over k-chunk PAIRS:
    half the PE cost of the bf16 row-sum matmul
  - PV via TWO fp8 DoubleRow matmuls per chunk-pair with v stored as an
    fp8 hi+lo pair (v_hi + v_lo == v to ~0.1%): bf16-parity cost, keeps
    v effectively exact while consuming the shared fp8 pt
  - causal masking via affine_select on the otherwise-idle Pool engine,
    zeroing the invalid triangle of the fp8 probs after exp
  - phase-2 software pipelining: scores/exp run ~3 chunks ahead of the
    DR consumers; out-proj tiles drip-fed between pairs to fill PE waits
"""

import sys

sys.path.insert(0, "/opt/trn_rl_repo")

import numpy as np
import ml_dtypes

import concourse.bass as bass
import concourse.tile as tile
from concourse import mybir
from concourse.bass import ts
from concourse.bass_utils import run_bass_kernel_spmd

# Provide antenv.axon_hooks (absent in this container) so trace=True can use
# the axon NTFF profiling path.
def _ensure_axon_hooks():
    import types

    try:
        from antenv import axon_hooks  # noqa: F401
        return
    except ImportError:
        pass
    import antenv

    mod = types.ModuleType("antenv.axon_hooks")
    mod._hook = None

    def set_axon_ntff_profile_hook(h):
        mod._hook = h

    def get_axon_ntff_profile_hook():
        return mod._hook

    mod.set_axon_ntff_profile_hook = set_axon_ntff_profile_hook
    mod.get_axon_ntff_profile_hook = get_axon_ntff_profile_hook
    sys.modules["antenv.axon_hooks"] = mod
    antenv.axon_hooks = mod
    try:
        from trn_agent_boot.trn_boot import _ntff_profile_via_ctypes

        hook = _ntff_profile_via_ctypes("/opt/axon/libaxon_pjrt.so")
        if hook is not None:
            mod._hook = hook
    except Exception:
        pass


_ensure_axon_hooks()

# ---------------------------------------------------------------------------
# This walrus build supports only ONE sync-wait command per instruction.
# TileContext's sem assignment can attach several waits to one instruction
# (and its exit drain aggregates many). Post-pass: hoist excess waits onto
# same-engine NoOps inserted immediately before the instruction -- the
# engine blocks on each wait in order, so semantics are identical.
MAX_WAITS_PER_INST = 1


_ALL_ENGINES = [
    mybir.EngineType.PE,
    mybir.EngineType.Activation,
    mybir.EngineType.DVE,
    mybir.EngineType.Pool,
    mybir.EngineType.SP,
]


def _split_sync_waits(nc):
    for f in nc.m.functions:
        for blk in f.blocks:
            new_insts = []
            for ins in blk.instructions:
                si = getattr(ins, "sync_info", None)
                lim = 1 if isinstance(ins, mybir.InstDrain) else MAX_WAITS_PER_INST
                if si is not None and si.on_wait and len(si.on_wait) > lim:
                    waits = list(si.on_wait)
                    keep = waits[:lim]
                    extra = waits[lim:]
                    # A drain with a big wait-set is the kernel-tail barrier:
                    # spread its waits across all engines so they resolve in
                    # parallel (the all-engine barrier right after joins them).
                    spread = (
                        isinstance(ins, mybir.InstDrain) and len(extra) > 4
                    )
                    for i, w in enumerate(extra):
                        eng = (
                            _ALL_ENGINES[i % len(_ALL_ENGINES)]
                            if spread
                            else ins.engine
                        )
                        nop = mybir.InstNoOp(
                            name=nc.get_next_instruction_name(),
                            sync_info=mybir.SyncInfo(on_wait=[w], on_update=[]),
                            bass_nofuse=True,
                            engine=eng,
                        )
                        new_insts.append(nop)
                    si.on_wait = keep
                new_insts.append(ins)
            if len(new_insts) != len(blk.instructions):
                blk.instructions = new_insts


# ---------------------------------------------------------------------------

B, T, C, H = 2, 2048, 2048, 16
D = C // H  # 128
ROPE_BASE = 10000.0
HG = 4  # head groups
HL = H // HG  # heads per core = 4
CL = HL * D  # local width = 512
P = 128
TB = 512  # token block
NTB = T // TB  # 4
KCH = T // P  # 16 k-chunks
QT = T // TB  # 4 q-tiles
SCALE = 1.0 / float(np.sqrt(D))
EXP_BIAS = -1.0  # exp shift; cancels in normalization, keeps pt under fp8e4 max
MASKNEG = -30000.0

F16 = mybir.dt.float16
FP8 = mybir.dt.float8e4
F32 = mybir.dt.float32
DR = mybir.MatmulPerfMode.DoubleRow
f16_np = np.float16
fp8_np = ml_dtypes.float8_e4m3


def _build_nc():
    nc = bass.Bass(trn_type="TRN2")
    xT = nc.declare_dram_parameter("xT", [C, T], F16, isOutput=False)
    wqkv = nc.declare_dram_parameter("wqkv", [C, 3 * CL], F16, isOutput=False)
    wout = nc.declare_dram_parameter("wout", [CL, C], F16, isOutput=False)
    tabs = nc.declare_dram_parameter("tabs", [2, P, T], F16, isOutput=False)
    out = nc.declare_dram_parameter("out", [T, C], F16, isOutput=True)

    xT_r = xT[:].rearrange("(ko p) t -> p ko t", p=P)  # [128,16,T]
    wqkv_r = wqkv[:].rearrange("(ko p) m -> p ko m", p=P)  # [128,16,1536]
    wout_r = wout[:].rearrange("(h p) n -> p h n", p=P)  # [128,4,2048]
    out_r = out[:].rearrange("(tc p) n -> tc p n", p=P)  # [16,128,2048]

    with tile.TileContext(nc) as tc:
        consts = tc.alloc_tile_pool(name="consts", bufs=1)
        persist = tc.alloc_tile_pool(name="persist", bufs=1)
        xpool = tc.alloc_tile_pool(name="xpool", bufs=2)
        prepool = tc.alloc_tile_pool(name="prepool", bufs=3)
        swppool = tc.alloc_tile_pool(name="swppool", bufs=4)
        ropepool = tc.alloc_tile_pool(name="ropepool", bufs=3)
        ppool = tc.alloc_tile_pool(name="ppool", bufs=4)
        vtmppool = tc.alloc_tile_pool(name="vtmppool", bufs=3)
        rpool = tc.alloc_tile_pool(name="rpool", bufs=2)
        outpool = tc.alloc_tile_pool(name="outpool", bufs=4)
        mm_psum = tc.alloc_tile_pool(name="mm_psum", bufs=2, space="PSUM")
        acc_psum = tc.alloc_tile_pool(name="acc_psum", bufs=3, space="PSUM")

        # ---- HAM warmup ----
        # PE is DMA-starved at start and its HAM clock gate starts cold
        # (1.2 GHz, ~3.4us to warm). Run dummy matmuls on scratch data during
        # the wait so real matmuls start at 2.4 GHz.
        warm_sb = prepool.tile([P, TB], F16, tag="pre")
        nc.vector.memset(warm_sb[:], 1.0)
        warm_ps = mm_psum.tile([P, 2, TB], F32, tag="mm")
        for _ in range(12):
            nc.tensor.matmul(
                warm_ps[:, 0, :], lhsT=warm_sb[:, :P], rhs=warm_sb[:],
                start=True, stop=True,
            )
        warm_out = prepool.tile([P, TB], F16, tag="pre")
        nc.scalar.copy(out=warm_out[:], in_=warm_ps[:, 0, :])

        # ---- constants ----
        # DMA issue order is program order; the first matmuls need only the
        # kc=0 slices of xT(tb=0) and W, so interleave those first.
        w_sb = consts.tile([P, KCH, 3 * CL], F16)  # 6MB
        xt0 = xpool.tile([P, KCH, TB], F16, tag="xt")
        for kc in range(KCH):
            nc.sync.dma_start(out=xt0[:, kc, :], in_=xT_r[:, kc, ts(0, TB)])
            nc.sync.dma_start(out=w_sb[:, kc, :], in_=wqkv_r[:, kc, :])
        bias_sb = consts.tile([P, 1], F32)
        nc.vector.memset(bias_sb[:], EXP_BIAS)
        cos_sb = consts.tile([P, T], F16)
        nc.sync.dma_start(out=cos_sb[:], in_=tabs[0])
        sin_sb = consts.tile([P, T], F16)
        nc.sync.dma_start(out=sin_sb[:], in_=tabs[1])
        ones_sb = consts.tile([P, 2, P], FP8)
        nc.vector.memset(ones_sb[:], 1.0)
        wo_sb = consts.tile([P, HL, C], F16)  # 2MB, loaded later (phase 3 use)

        # ---- persistent activations ----
        qT_sb = persist.tile([P, HL, T], F16)  # 2MB
        kT_sb = persist.tile([P, HL, T], F16)  # 2MB
        vhi_sb = persist.tile([P, KCH, CL], FP8)  # 1MB
        vlo_sb = persist.tile([P, KCH, CL], FP8)  # 1MB
        oT_sb = persist.tile([P, HL, T], F16)  # 2MB

        # ================= Phase 1: QKV + RoPE =================
        for tb in range(NTB):
            if tb == 0:
                xt = xt0
            else:
                xt = xpool.tile([P, KCH, TB], F16, tag="xt")
                nc.sync.dma_start(out=xt[:], in_=xT_r[:, :, ts(tb, TB)])

            # two accumulation chains share each 2-bank psum tile
            chain_tile = [None]

            def chain_psum(ch):
                if ch % 2 == 0:
                    chain_tile[0] = mm_psum.tile(
                        [P, 2, TB], F32, tag="mm", name="qp"
                    )
                return chain_tile[0][:, ch % 2, :]

            # qT / kT in [D, token] layout + RoPE
            for m in range(2 * HL):  # 0..3 q heads, 4..7 k heads
                qp = chain_psum(m)
                for kc in range(KCH):
                    nc.tensor.matmul(
                        qp[:],
                        lhsT=w_sb[:, kc, ts(m, P)],
                        rhs=xt[:, kc, :],
                        start=(kc == 0),
                        stop=(kc == KCH - 1),
                    )
                pre = prepool.tile([P, TB], F16, tag="pre")
                nc.scalar.copy(out=pre[:], in_=qp[:])
                del qp
                # swap partition halves via SBUF->SBUF DMA (DVE cannot cross
                # partitions; walrus requires equal base partitions for TT)
                h64 = D // 2
                swp = swppool.tile([P, TB], F16)
                nc.sync.dma_start(out=swp[0:h64], in_=pre[h64 : 2 * h64])
                nc.sync.dma_start(out=swp[h64 : 2 * h64], in_=pre[0:h64])
                ta = ropepool.tile([P, TB], F16, tag="ta")
                tb_ = ropepool.tile([P, TB], F16, tag="tb")
                # rope = pre * cosF + swap(pre) * sinS   (sinS = [-sin; +sin])
                nc.vector.tensor_mul(ta[:], pre[:], cos_sb[:, ts(tb, TB)])
                nc.vector.tensor_mul(tb_[:], swp[:], sin_sb[:, ts(tb, TB)])
                dest = (
                    qT_sb[:, m, ts(tb, TB)] if m < HL else kT_sb[:, m - HL, ts(tb, TB)]
                )
                nc.vector.tensor_add(dest[:], ta[:], tb_[:])

            # V in natural [token, D] layout; one fast ACT copy frees the
            # psum slot, then the fp8 hi+lo split happens SBUF-side on DVE
            for tsc in range(TB // P):
                vp = chain_psum(2 * HL + tsc)
                for kc in range(KCH):
                    nc.tensor.matmul(
                        vp[:],
                        lhsT=xt[:, kc, ts(tsc, P)],
                        rhs=w_sb[:, kc, 2 * CL : 3 * CL],
                        start=(kc == 0),
                        stop=(kc == KCH - 1),
                    )
                vslot = tb * (TB // P) + tsc
                v16 = vtmppool.tile([P, TB], F16, tag="v16")
                nc.scalar.copy(out=v16[:], in_=vp[:])
                nc.vector.tensor_copy(out=vhi_sb[:, vslot, :], in_=v16[:])
                nc.vector.tensor_sub(
                    vlo_sb[:, vslot, :], v16[:], vhi_sb[:, vslot, :]
                )

        # ================= Phase 2+3: attention (qt-outer) + out-proj =====
        # this load overlaps phase 1/2 compute
        nc.sync.dma_start(out=wo_sb[:], in_=wout_r)

        out_units = []  # (tcc, ncc) out-proj tiles ready to emit

        def emit_out_unit(flush=False):
            if not out_units:
                return
            tcc, ncc = out_units.pop(0)
            if flush:
                # after the last attention head the acc slots are free;
                # using them lets the final 16 tiles pipeline 3-deep
                outp = acc_psum.tile([P, TB], F32, tag="acc", name="outp_f")
            else:
                outp = mm_psum.tile([P, TB], F32, tag="outp", bufs=1)
            for h in range(HL):
                nc.tensor.matmul(
                    outp[:],
                    lhsT=oT_sb[:, h, ts(tcc, P)],
                    rhs=wo_sb[:, h, ts(ncc, TB)],
                    start=(h == 0),
                    stop=(h == HL - 1),
                )
            ot = outpool.tile([P, TB], F16)
            nc.vector.tensor_copy(out=ot[:], in_=outp[:])
            nc.sync.dma_start(out=out_r[tcc, :, ts(ncc, TB)], in_=ot[:])

        def emit_attention(qt, h):
            op = acc_psum.tile([P, TB], F32, tag="acc")
            lp = acc_psum.tile([P, TB], F32, tag="acc")
            nkc = (qt + 1) * (TB // P)
            npair = nkc // 2
            pts = {}
            sps = {}

            def pair_geom(p_):
                off_e = 2 * p_ - qt * (TB // P)
                qs = max(off_e, 0) * P
                return off_e >= 0, qs, TB - qs

            def produce(c):
                p_ = c // 2
                diag, qs, W = pair_geom(p_)
                if c % 2 == 0:
                    sps[p_] = mm_psum.tile([P, 2, TB], F32, tag="mm", name="sp")
                    pts[p_] = ppool.tile([P, 2, TB], FP8, name="pt")
                sp = sps[p_]
                nc.tensor.matmul(
                    sp[:, c % 2, :W],
                    lhsT=kT_sb[:, h, ts(c, P)],
                    rhs=qT_sb[:, h, qt * TB + qs : (qt + 1) * TB],
                    start=True,
                    stop=True,
                )
                if c % 2 == 1:
                    # one exp covers the whole chunk-pair
                    nc.scalar.activation(
                        out=pts[p_][:, :, :W],
                        in_=sp[:, :, :W],
                        func=mybir.ActivationFunctionType.Exp,
                        scale=SCALE,
                        bias=bias_sb[:],
                    )
                    sps.pop(p_)
                    if diag:
                        # zero the causally-invalid triangle on the idle Pool
                        # engine: keep where (c_local - 128*subtile - k) >= 0
                        nc.gpsimd.affine_select(
                            out=pts[p_][:, :, :W],
                            in_=pts[p_][:, :, :W],
                            pattern=[[-P, 2], [1, W]],
                            compare_op=mybir.AluOpType.is_ge,
                            fill=0.0,
                            base=0,
                            channel_multiplier=-1,
                        )

            def consume(p_):
                _, qs, W = pair_geom(p_)
                kc = 2 * p_
                pt = pts.pop(p_)
                last = p_ == npair - 1
                nc.tensor.matmul(
                    op[:, qs:],
                    lhsT=vhi_sb[:, kc : kc + 2, ts(h, P)],
                    rhs=pt[:, :, :W],
                    start=(p_ == 0),
                    stop=False,
                    perf_mode=DR,
                )
                nc.tensor.matmul(
                    op[:, qs:],
                    lhsT=vlo_sb[:, kc : kc + 2, ts(h, P)],
                    rhs=pt[:, :, :W],
                    start=False,
                    stop=last,
                    perf_mode=DR,
                )
                nc.tensor.matmul(
                    lp[:, qs:],
                    lhsT=ones_sb[:],
                    rhs=pt[:, :, :W],
                    start=(p_ == 0),
                    stop=last,
                    perf_mode=DR,
                )

            # scores/exp run ~3 chunks ahead of the DR consumers so exp
            # latency hides behind PE work; out-proj drips fill the rest
            done = 0
            for c in range(nkc):
                produce(c)
                if c >= 3 and c % 2 == 1:
                    consume(done)
                    done += 1
                    emit_out_unit()
            while done < npair:
                consume(done)
                done += 1

            # r = 1/l = exp(-ln(l)); fold into oT
            lt = rpool.tile([P, TB], F32, tag="lt")
            nc.scalar.activation(
                out=lt[:], in_=lp[:], func=mybir.ActivationFunctionType.Ln
            )
            rt = rpool.tile([P, TB], F32, tag="rt")
            nc.scalar.activation(
                out=rt[:],
                in_=lt[:],
                func=mybir.ActivationFunctionType.Exp,
                scale=-1.0,
            )
            nc.vector.tensor_mul(oT_sb[:, h, ts(qt, TB)], op[:], rt[:])

        for qt in range(QT):
            for h in range(HL):
                emit_attention(qt, h)
                emit_out_unit()
                emit_out_unit()
            out_units.extend(
                (tcc, ncc)
                for tcc in range(qt * (TB // P), (qt + 1) * (TB // P))
                for ncc in range(C // TB)
            )
        while out_units:
            emit_out_unit(flush=True)

        for pool in (
            acc_psum,
            mm_psum,
            outpool,
            rpool,
            vtmppool,
            ppool,
            ropepool,
            swppool,
            prepool,
            xpool,
            persist,
            consts,
        ):
            pool.release()

    _split_sync_waits(nc)
    return nc


def _host_inputs(x, W_qkv, W_out):
    """Build per-core input maps. Core j: batch j//HG, head-group j%HG."""
    perm = np.concatenate([np.arange(0, D, 2), np.arange(1, D, 2)])  # deinterleave

    # rope tables in de-interleaved layout: rows [0:64]=even-dim freq, dup below
    inv = 1.0 / (ROPE_BASE ** (np.arange(0, D, 2, dtype=np.float32) / D))  # [64]
    ang = np.arange(T, dtype=np.float32)[None, :] * inv[:, None]  # [64, T]
    cosF = np.concatenate([np.cos(ang), np.cos(ang)], axis=0)  # [128, T]
    sinS = np.concatenate([-np.sin(ang), np.sin(ang)], axis=0)  # sign folded
    tabs = np.stack([cosF, sinS]).astype(f16_np)  # [2,128,T]

    in_maps = []
    for j in range(8):
        b, hg = j // HG, j % HG
        xTb = np.ascontiguousarray(x[b].T).astype(f16_np)  # [C, T]
        cols = []
        for part in range(2):  # q, k with permuted D
            for h in range(HL):
                base = part * C + (hg * HL + h) * D
                cols.append(W_qkv[:, base + perm])
        for h in range(HL):  # v natural
            base = 2 * C + (hg * HL + h) * D
            cols.append(W_qkv[:, base : base + D])
        wq = np.concatenate(cols, axis=1).astype(f16_np)  # [C, 3*CL]
        wo = W_out[hg * CL : (hg + 1) * CL, :].astype(f16_np)  # [CL, C]
        in_maps.append({"xT": xTb, "wqkv": wq, "wout": wo, "tabs": tabs})
    return in_maps


def kernel(x, W_qkv, W_out, _trace=False, _tmpdir=None):
    x = np.asarray(x, dtype=np.float32)
    W_qkv = np.asarray(W_qkv, dtype=np.float32)
    W_out = np.asarray(W_out, dtype=np.float32)

    nc = _build_nc()
    in_maps = _host_inputs(x, W_qkv, W_out)
    res = run_bass_kernel_spmd(
        nc, in_maps, core_ids=list(range(8)), trace=_trace, tmpdir=_tmpdir
    )

    out = np.zeros((B, T, C), dtype=np.float32)
    for j in range(8):
        out[j // HG] += res.results[j]["out"].astype(np.float32)
    if _trace:
        return out, res
    return out


# revision 30
# speedup vs baseline: 1.0813x; 1.0005x over previous
"""Trainium2 Bass kernel for MultiHeadAttentionWithRope.

Problem: B=2, T=2048, C=2048, H=16 heads, D=128 head_dim, fp32 I/O.
  qkv = x @ W_qkv; q,k -> RoPE (adjacent-pair, torchtune) -> causal SDPA
  -> out = o @ W_out
Sharding (8 cores): 2 batches x 4 head-groups (4 heads each); host sums the
4 row-shard partials of W_out per batch.

Numerics/performance scheme (per-core PE time is the binding constraint):
  - all dense GEMMs in fp16 (same PE rate as bf16, 8x less noise)
  - softmax probs pt stored fp8e4 with a constant exp bias (shift cancels in
    the l-normalization): the ONLY tensor whose fp8 noise the output metric
    tolerates (renormalization cancels the dominant-prob error)
  - row-sums l via an all-ones fp8 DoubleRow matmul over k-chunk PAIRS:
    half the PE cost of the bf16 row-sum matmul
  - PV via TWO fp8 DoubleRow matmuls per chunk-pair with v stored as an
    fp8 hi+lo pair (v_hi + v_lo == v to ~0.1%): bf16-parity cost, keeps
    v effectively exact while consuming the shared fp8 pt
  - causal masking via affine_select on the otherwise-idle Pool engine,
    zeroing the invalid triangle of the fp8 probs after exp
  - phase-2 software pipelining: scores/exp run ~3 chunks ahead of the
    DR consumers; out-proj tiles drip-fed between pairs to fill PE waits
"""

import sys

sys.path.insert(0, "/opt/trn_rl_repo")

import numpy as np
import ml_dtypes

import concourse.bass as bass
import concourse.tile as tile
from concourse import mybir
from concourse.bass import ts
from concourse.bass_utils import run_bass_kernel_spmd

# Provide antenv.axon_hooks (absent in this container) so trace=True can use
# the axon NTFF profiling path.
def _ensure_axon_hooks():
    import types

    try:
        from antenv import axon_hooks  # noqa: F401
        return
    except ImportError:
        pass
    import antenv

    mod = types.ModuleType("antenv.axon_hooks")
    mod._hook = None

    def set_axon_ntff_profile_hook(h):
        mod._hook = h

    def get_axon_ntff_profile_hook():
        return mod._hook

    mod.set_axon_ntff_profile_hook = set_axon_ntff_profile_hook
    mod.get_axon_ntff_profile_hook = get_axon_ntff_profile_hook
    sys.modules["antenv.axon_hooks"] = mod
    antenv.axon_hooks = mod
    try:
        from trn_agent_boot.trn_boot import _ntff_profile_via_ctypes

        hook = _ntff_profile_via_ctypes("/opt/axon/libaxon_pjrt.so")
        if hook is not None:
            mod._hook = hook
    except Exception:
        pass


_ensure_axon_hooks()

# ---------------------------------------------------------------------------
# This walrus build supports only ONE sync-wait command per instruction.
# TileContext's sem assignment can attach several waits to one instruction
# (and its exit drain aggregates many). Post-pass: hoist excess waits onto
# same-engine NoOps inserted immediately before the instruction -- the
# engine blocks on each wait in order, so semantics are identical.
MAX_WAITS_PER_INST = 1


_ALL_ENGINES = [
    mybir.EngineType.PE,
    mybir.EngineType.Activation,
    mybir.EngineType.DVE,
    mybir.EngineType.Pool,
    mybir.EngineType.SP,
]


def _split_sync_waits(nc):
    for f in nc.m.functions:
        for blk in f.blocks:
            new_insts = []
            for ins in blk.instructions:
                si = getattr(ins, "sync_info", None)
                lim = 1 if isinstance(ins, mybir.InstDrain) else MAX_WAITS_PER_INST
                if si is not None and si.on_wait and len(si.on_wait) > lim:
                    waits = list(si.on_wait)
                    keep = waits[:lim]
                    extra = waits[lim:]
                    # A drain with a big wait-set is the kernel-tail barrier:
                    # spread its waits across all engines so they resolve in
                    # parallel (the all-engine barrier right after joins them).
                    spread = (
                        isinstance(ins, mybir.InstDrain) and len(extra) > 4
                    )
                    for i, w in enumerate(extra):
                        eng = (
                            _ALL_ENGINES[i % len(_ALL_ENGINES)]
                            if spread
                            else ins.engine
                        )
                        nop = mybir.InstNoOp(
                            name=nc.get_next_instruction_name(),
                            sync_info=mybir.SyncInfo(on_wait=[w], on_update=[]),
                            bass_nofuse=True,
                            engine=eng,
                        )
                        new_insts.append(nop)
                    si.on_wait = keep
                new_insts.append(ins)
            if len(new_insts) != len(blk.instructions):
                blk.instructions = new_insts


# ---------------------------------------------------------------------------

B, T, C, H = 2, 2048, 2048, 16
D = C // H  # 128
ROPE_BASE = 10000.0
HG = 4  # head groups
HL = H // HG  # heads per core = 4
CL = HL * D  # local width = 512
P = 128
TB = 512  # token block
NTB = T // TB  # 4
KCH = T // P  # 16 k-chunks
QT = T // TB  # 4 q-tiles
SCALE = 1.0 / float(np.sqrt(D))
EXP_BIAS = -1.0  # exp shift; cancels in normalization, keeps pt under fp8e4 max
MASKNEG = -30000.0

F16 = mybir.dt.float16
FP8 = mybir.dt.float8e4
F32 = mybir.dt.float32
DR = mybir.MatmulPerfMode.DoubleRow
f16_np = np.float16
fp8_np = ml_dtypes.float8_e4m3


def _build_nc():
    nc = bass.Bass(trn_type="TRN2")
    xT = nc.declare_dram_parameter("xT", [C, T], F16, isOutput=False)
    wqkv = nc.declare_dram_parameter("wqkv", [C, 3 * CL], F16, isOutput=False)
    wout = nc.declare_dram_parameter("wout", [CL, C], F16, isOutput=False)
    tabs = nc.declare_dram_parameter("tabs", [2, P, T], F16, isOutput=False)
    out = nc.declare_dram_parameter("out", [T, C], F16, isOutput=True)

    xT_r = xT[:].rearrange("(ko p) t -> p ko t", p=P)  # [128,16,T]
    wqkv_r = wqkv[:].rearrange("(ko p) m -> p ko m", p=P)  # [128,16,1536]
    wout_r = wout[:].rearrange("(h p) n -> p h n", p=P)  # [128,4,2048]
    out_r = out[:].rearrange("(tc p) n -> tc p n", p=P)  # [16,128,2048]

    with tile.TileContext(nc) as tc:
        consts = tc.alloc_tile_pool(name="consts", bufs=1)
        persist = tc.alloc_tile_pool(name="persist", bufs=1)
        xpool = tc.alloc_tile_pool(name="xpool", bufs=2)
        prepool = tc.alloc_tile_pool(name="prepool", bufs=3)
        swppool = tc.alloc_tile_pool(name="swppool", bufs=4)
        ropepool = tc.alloc_tile_pool(name="ropepool", bufs=3)
        ppool = tc.alloc_tile_pool(name="ppool", bufs=4)
        vtmppool = tc.alloc_tile_pool(name="vtmppool", bufs=3)
        rpool = tc.alloc_tile_pool(name="rpool", bufs=2)
        outpool = tc.alloc_tile_pool(name="outpool", bufs=4)
        mm_psum = tc.alloc_tile_pool(name="mm_psum", bufs=2, space="PSUM")
        acc_psum = tc.alloc_tile_pool(name="acc_psum", bufs=3, space="PSUM")

        # ---- HAM warmup ----
        # PE is DMA-starved at start and its HAM clock gate starts cold
        # (1.2 GHz, ~3.4us to warm). Run dummy matmuls on scratch data during
        # the wait so real matmuls start at 2.4 GHz.
        warm_sb = prepool.tile([P, TB], F16, tag="pre")
        nc.vector.memset(warm_sb[:], 1.0)
        warm_ps = mm_psum.tile([P, 2, TB], F32, tag="mm")
        for _ in range(12):
            nc.tensor.matmul(
                warm_ps[:, 0, :], lhsT=warm_sb[:, :P], rhs=warm_sb[:],
                start=True, stop=True,
            )
        warm_out = prepool.tile([P, TB], F16, tag="pre")
        nc.scalar.copy(out=warm_out[:], in_=warm_ps[:, 0, :])

        # ---- constants ----
        # DMA issue order is program order; the first matmuls need only the
        # kc=0 slices of xT(tb=0) and W, so interleave those first.
        w_sb = consts.tile([P, KCH, 3 * CL], F16)  # 6MB
        xt0 = xpool.tile([P, KCH, TB], F16, tag="xt")
        for kc in range(KCH):
            nc.sync.dma_start(out=xt0[:, kc, :], in_=xT_r[:, kc, ts(0, TB)])
            nc.sync.dma_start(out=w_sb[:, kc, :], in_=wqkv_r[:, kc, :])
        bias_sb = consts.tile([P, 1], F32)
        nc.vector.memset(bias_sb[:], EXP_BIAS)
        cos_sb = consts.tile([P, T], F16)
        nc.sync.dma_start(out=cos_sb[:], in_=tabs[0])
        sin_sb = consts.tile([P, T], F16)
        nc.sync.dma_start(out=sin_sb[:], in_=tabs[1])
        ones_sb = consts.tile([P, 2, P], FP8)
        nc.vector.memset(ones_sb[:], 1.0)
        wo_sb = consts.tile([P, HL, C], F16)  # 2MB, loaded later (phase 3 use)

        # ---- persistent activations ----
        qT_sb = persist.tile([P, HL, T], F16)  # 2MB
        kT_sb = persist.tile([P, HL, T], F16)  # 2MB
        vhi_sb = persist.tile([P, KCH, CL], FP8)  # 1MB
        vlo_sb = persist.tile([P, KCH, CL], FP8)  # 1MB
        oT_sb = persist.tile([P, HL, T], F16)  # 2MB

        # ================= Phase 1: QKV + RoPE =================
        for tb in range(NTB):
            if tb == 0:
                xt = xt0
            else:
                xt = xpool.tile([P, KCH, TB], F16, tag="xt")
                nc.sync.dma_start(out=xt[:], in_=xT_r[:, :, ts(tb, TB)])

            # two accumulation chains share each 2-bank psum tile
            chain_tile = [None]

            def chain_psum(ch):
                if ch % 2 == 0:
                    chain_tile[0] = mm_psum.tile(
                        [P, 2, TB], F32, tag="mm", name="qp"
                    )
                return chain_tile[0][:, ch % 2, :]

            # qT / kT in [D, token] layout + RoPE
            for m in range(2 * HL):  # 0..3 q heads, 4..7 k heads
                qp = chain_psum(m)
                for kc in range(KCH):
                    nc.tensor.matmul(
                        qp[:],
                        lhsT=w_sb[:, kc, ts(m, P)],
                        rhs=xt[:, kc, :],
                        start=(kc == 0),
                        stop=(kc == KCH - 1),
                    )
                pre = prepool.tile([P, TB], F16, tag="pre")
                nc.scalar.copy(out=pre[:], in_=qp[:])
                del qp
                # swap partition halves via SBUF->SBUF DMA (DVE cannot cross
                # partitions; walrus requires equal base partitions for TT)
                h64 = D // 2
                swp = swppool.tile([P, TB], F16)
                nc.sync.dma_start(out=swp[0:h64], in_=pre[h64 : 2 * h64])
                nc.sync.dma_start(out=swp[h64 : 2 * h64], in_=pre[0:h64])
                ta = ropepool.tile([P, TB], F16, tag="ta")
                tb_ = ropepool.tile([P, TB], F16, tag="tb")
                # rope = pre * cosF + swap(pre) * sinS   (sinS = [-sin; +sin])
                nc.vector.tensor_mul(ta[:], pre[:], cos_sb[:, ts(tb, TB)])
                nc.vector.tensor_mul(tb_[:], swp[:], sin_sb[:, ts(tb, TB)])
                dest = (
                    qT_sb[:, m, ts(tb, TB)] if m < HL else kT_sb[:, m - HL, ts(tb, TB)]
                )
                nc.vector.tensor_add(dest[:], ta[:], tb_[:])

            # V in natural [token, D] layout; one fast ACT copy frees the
            # psum slot, then the fp8 hi+lo split happens SBUF-side on DVE
            for tsc in range(TB // P):
                vp = chain_psum(2 * HL + tsc)
                for kc in range(KCH):
                    nc.tensor.matmul(
                        vp[:],
                        lhsT=xt[:, kc, ts(tsc, P)],
                        rhs=w_sb[:, kc, 2 * CL : 3 * CL],
                        start=(kc == 0),
                        stop=(kc == KCH - 1),
                    )
                vslot = tb * (TB // P) + tsc
                v16 = vtmppool.tile([P, TB], F16, tag="v16")
                nc.scalar.copy(out=v16[:], in_=vp[:])
                nc.vector.tensor_copy(out=vhi_sb[:, vslot, :], in_=v16[:])
                nc.vector.tensor_sub(
                    vlo_sb[:, vslot, :], v16[:], vhi_sb[:, vslot, :]
                )

        # ================= Phase 2+3: attention (qt-outer) + out-proj =====
        # this load overlaps phase 1/2 compute
        nc.sync.dma_start(out=wo_sb[:], in_=wout_r)

        out_units = []  # (tcc, ncc) out-proj tiles ready to emit

        def emit_out_unit(flush=False):
            if not out_units:
                return
            tcc, ncc = out_units.pop(0)
            if flush:
                # after the last attention head the acc slots are free;
                # using them lets the final 16 tiles pipeline 3-deep
                outp = acc_psum.tile([P, TB], F32, tag="acc", name="outp_f")
            else:
                outp = mm_psum.tile([P, TB], F32, tag="outp", bufs=1)
            for h in range(HL):
                nc.tensor.matmul(
                    outp[:],
                    lhsT=oT_sb[:, h, ts(tcc, P)],
                    rhs=wo_sb[:, h, ts(ncc, TB)],
                    start=(h == 0),
                    stop=(h == HL - 1),
                )
            ot = outpool.tile([P, TB], F16)
            nc.vector.tensor_copy(out=ot[:], in_=outp[:])
            nc.sync.dma_start(out=out_r[tcc, :, ts(ncc, TB)], in_=ot[:])

        def emit_attention(qt, h):
            op = acc_psum.tile([P, TB], F32, tag="acc")
            lp = acc_psum.tile([P, TB], F32, tag="acc")
            nkc = (qt + 1) * (TB // P)
            npair = nkc // 2
            pts = {}
            sps = {}

            def pair_geom(p_):
                off_e = 2 * p_ - qt * (TB // P)
                qs = max(off_e, 0) * P
                return off_e >= 0, qs, TB - qs

            def produce(c):
                p_ = c // 2
                diag, qs, W = pair_geom(p_)
                if c % 2 == 0:
                    sps[p_] = mm_psum.tile([P, 2, TB], F32, tag="mm", name="sp")
                    pts[p_] = ppool.tile([P, 2, TB], FP8, name="pt")
                sp = sps[p_]
                nc.tensor.matmul(
                    sp[:, c % 2, :W],
                    lhsT=kT_sb[:, h, ts(c, P)],
                    rhs=qT_sb[:, h, qt * TB + qs : (qt + 1) * TB],
                    start=True,
                    stop=True,
                )
                if c % 2 == 1:
                    # one exp covers the whole chunk-pair
                    nc.scalar.activation(
                        out=pts[p_][:, :, :W],
                        in_=sp[:, :, :W],
                        func=mybir.ActivationFunctionType.Exp,
                        scale=SCALE,
                        bias=bias_sb[:],
                    )
                    sps.pop(p_)
                    if diag:
                        # zero the causally-invalid triangle on the idle Pool
                        # engine: keep where (c_local - 128*subtile - k) >= 0
                        nc.gpsimd.affine_select(
                            out=pts[p_][:, :, :W],
                            in_=pts[p_][:, :, :W],
                            pattern=[[-P, 2], [1, W]],
                            compare_op=mybir.AluOpType.is_ge,
                            fill=0.0,
                            base=0,
                            channel_multiplier=-1,
                        )

            def consume(p_):
                _, qs, W = pair_geom(p_)
                kc = 2 * p_
                pt = pts.pop(p_)
                last = p_ == npair - 1
                nc.tensor.matmul(
                    op[:, qs:],
                    lhsT=vhi_sb[:, kc : kc + 2, ts(h, P)],
                    rhs=pt[:, :, :W],
                    start=(p_ == 0),
                    stop=False,
                    perf_mode=DR,
                )
                nc.tensor.matmul(
                    op[:, qs:],
                    lhsT=vlo_sb[:, kc : kc + 2, ts(h, P)],
                    rhs=pt[:, :, :W],
                    start=False,
                    stop=last,
                    perf_mode=DR,
                )
                nc.tensor.matmul(
                    lp[:, qs:],
                    lhsT=ones_sb[:],
                    rhs=pt[:, :, :W],
                    start=(p_ == 0),
                    stop=last,
                    perf_mode=DR,
                )

            # scores/exp run ~3 chunks ahead of the DR consumers so exp
            # latency hides behind PE work; out-proj drips fill the rest.
            # The previous head's 1/l epilogue is emitted AFTER this head's
            # first exp so it doesn't delay it in the in-order ACT queue.
            done = 0
            for c in range(nkc):
                produce(c)
                if c == 1 and pending_epilogue[0] is not None:
                    pending_epilogue[0]()
                    pending_epilogue[0] = None
                if c >= 3 and c % 2 == 1:
                    consume(done)
                    done += 1
                    emit_out_unit()
            while done < npair:
                consume(done)
                done += 1

            def epilogue():
                # r = 1/l = exp(-ln(l)); fold into oT
                lt = rpool.tile([P, TB], F32, tag="lt")
                nc.scalar.activation(
                    out=lt[:], in_=lp[:], func=mybir.ActivationFunctionType.Ln
                )
                rt = rpool.tile([P, TB], F32, tag="rt")
                nc.scalar.activation(
                    out=rt[:],
                    in_=lt[:],
                    func=mybir.ActivationFunctionType.Exp,
                    scale=-1.0,
                )
                nc.vector.tensor_mul(oT_sb[:, h, ts(qt, TB)], op[:], rt[:])

            pending_epilogue[0] = epilogue

        pending_epilogue = [None]
        for qt in range(QT):
            for h in range(HL):
                emit_attention(qt, h)
                emit_out_unit()
                emit_out_unit()
            out_units.extend(
                (tcc, ncc)
                for tcc in range(qt * (TB // P), (qt + 1) * (TB // P))
                for ncc in range(C // TB)
            )
        if pending_epilogue[0] is not None:
            pending_epilogue[0]()
            pending_epilogue[0] = None
        while out_units:
            emit_out_unit(flush=True)

        for pool in (
            acc_psum,
            mm_psum,
            outpool,
            rpool,
            vtmppool,
            ppool,
            ropepool,
            swppool,
            prepool,
            xpool,
            persist,
            consts,
        ):
            pool.release()

    _split_sync_waits(nc)
    return nc


def _host_inputs(x, W_qkv, W_out):
    """Build per-core input maps. Core j: batch j//HG, head-group j%HG."""
    perm = np.concatenate([np.arange(0, D, 2), np.arange(1, D, 2)])  # deinterleave

    # rope tables in de-interleaved layout: rows [0:64]=even-dim freq, dup below
    inv = 1.0 / (ROPE_BASE ** (np.arange(0, D, 2, dtype=np.float32) / D))  # [64]
    ang = np.arange(T, dtype=np.float32)[None, :] * inv[:, None]  # [64, T]
    cosF = np.concatenate([np.cos(ang), np.cos(ang)], axis=0)  # [128, T]
    sinS = np.concatenate([-np.sin(ang), np.sin(ang)], axis=0)  # sign folded
    tabs = np.stack([cosF, sinS]).astype(f16_np)  # [2,128,T]

    in_maps = []
    for j in range(8):
        b, hg = j // HG, j % HG
        xTb = np.ascontiguousarray(x[b].T).astype(f16_np)  # [C, T]
        cols = []
        for part in range(2):  # q, k with permuted D
            for h in range(HL):
                base = part * C + (hg * HL + h) * D
                cols.append(W_qkv[:, base + perm])
        for h in range(HL):  # v natural
            base = 2 * C + (hg * HL + h) * D
            cols.append(W_qkv[:, base : base + D])
        wq = np.concatenate(cols, axis=1).astype(f16_np)  # [C, 3*CL]
        wo = W_out[hg * CL : (hg + 1) * CL, :].astype(f16_np)  # [CL, C]
        in_maps.append({"xT": xTb, "wqkv": wq, "wout": wo, "tabs": tabs})
    return in_maps


def kernel(x, W_qkv, W_out, _trace=False, _tmpdir=None):
    x = np.asarray(x, dtype=np.float32)
    W_qkv = np.asarray(W_qkv, dtype=np.float32)
    W_out = np.asarray(W_out, dtype=np.float32)

    nc = _build_nc()
    in_maps = _host_inputs(x, W_qkv, W_out)
    res = run_bass_kernel_spmd(
        nc, in_maps, core_ids=list(range(8)), trace=_trace, tmpdir=_tmpdir
    )

    out = np.zeros((B, T, C), dtype=np.float32)
    for j in range(8):
        out[j // HG] += res.results[j]["out"].astype(np.float32)
    if _trace:
        return out, res
    return out
